# revision 35
# baseline (speedup 1.0000x reference)
"""Bass/Trainium2 kernel for PixelDSNTDistanceDoublePredict loss.

Full inputs: input [32,2,512,512] f32, target [32,2,512,512] f32.
Data-parallel over batch: core k handles batches [4k, 4k+4) = 8 heatmaps.

Per heatmap (viewed as [p=128, (t,w)=2048], h = t*128+p):
  softmax path:  E = exp(x) on ACT.  Stage-1: 16 matmuls with lhsT =
    E[:, 128j:128j+128] (weights) and rhs = G[128,2] (col0=1,
    col1=(p+1)/512) -> P1[m, 2j+c] = sum_p E[p,128j+m]*G[p,c] in PSUM.
    Stage-2: lhsT = P1 (SBUF copy), rhs = G -> out2[2j+c, n] with
    n in {sum_m, sum_m*(m+1)/512}.  Host reconstructs
    S = sum e, Sx = sum e*(w+1)/512, Sy = sum e*(h+1)/512 from out2
    (x/y grids decompose into i-part + j-part / p-part + t-part).
  argmax path:  DVE max (top-8 per partition) + max_index -> per-row
    max value and first-occurrence index; host resolves the global
    argmax exactly (fp32 comparisons are exact).

Host finishes the scalar loss math in float64.
"""

import numpy as np

import concourse.bass as bass
import concourse.mybir as mybir
import concourse.tile as tile
from concourse.bass_utils import run_bass_kernel_spmd

N_CORES = 8
B, C, H, W = 32, 2, 512, 512
B_LOC = B // N_CORES            # 4 batches per core
HM = B_LOC * C                  # 8 heatmaps per core
P = 128                         # partitions
TBLK = H // P                   # 4 row-blocks per heatmap
FREE = TBLK * W                 # 2048 free elements per partition
NCH = FREE // P                 # 16 weight chunks per heatmap

_PROGRAM = None


def _split_multi_waits(nc):
    """walrus can encode at most one sync wait on compute instructions;
    move extra waits onto a preceding EventSemaphore on the same queue."""
    for f in nc.m.functions:
        for bb in f.blocks:
            new = []
            for ins in bb.instructions:
                si = ins.sync_info
                if (
                    si is not None
                    and si.on_wait
                    and len(si.on_wait) > 1
                    and ins.opcode != "EventSemaphore"
                ):
                    for wi, w in enumerate(si.on_wait[:-1]):
                        ev = mybir.InstEventSemaphore(
                            name=f"{ins.name}-wsplit{wi}",
                            engine=ins.engine,
                            sync_info=mybir.SyncInfo(on_wait=[w], on_update=[]),
                            ins=[],
                            outs=[],
                        )
                        new.append(ev)
                    ins.sync_info = mybir.SyncInfo(
                        on_wait=[si.on_wait[-1]], on_update=list(si.on_update)
                    )
                new.append(ins)
            bb.instructions = new


def _build_program():
    nc = bass.Bass()
    x = nc.dram_tensor("x", [HM, H, W], mybir.dt.float32, kind="ExternalInput")
    t = nc.dram_tensor("t", [HM, H, W], mybir.dt.float32, kind="ExternalInput")
    # per-chunk rhs constants: for chunk j cols [1, (p+1)/W, (j%4)/4, (j//4)/4]
    # (all exactly representable in bf16)
    g = nc.dram_tensor("g", [P, NCH, 4], mybir.dt.bfloat16, kind="ExternalInput")
    sums = nc.dram_tensor("sums", [P, 4 * HM], mybir.dt.float32, kind="ExternalOutput")
    maxv = nc.dram_tensor("maxv", [P, HM], mybir.dt.float32, kind="ExternalOutput")
    maxi = nc.dram_tensor("maxi", [P, HM], mybir.dt.uint32, kind="ExternalOutput")

    with tile.TileContext(nc) as tc:
        with (
            tc.tile_pool(name="consts", bufs=1) as consts,
            tc.tile_pool(name="xin", bufs=HM) as xin,
            tc.tile_pool(name="tin", bufs=HM) as tin,
            tc.tile_pool(name="etp", bufs=HM) as etp,
            tc.tile_pool(name="small", bufs=4) as small,
            tc.tile_pool(name="ps1", bufs=4, space="PSUM") as ps1,
            tc.tile_pool(name="stage", bufs=1) as stage,
        ):
            # Bounce g through an ACT copy: the first stage-1 matmul then
            # depends on gt via the ACT semaphore only (merged with its
            # exp dependency).  A direct DMA dep would give the matmul two
            # sync waits, which walrus cannot encode on LDWEIGHTS.
            gt_raw = consts.tile([P, NCH, 4], mybir.dt.bfloat16)
            nc.sync.dma_start(out=gt_raw, in_=g[:, :, :])
            gt = consts.tile([P, NCH, 4], mybir.dt.bfloat16)
            nc.scalar.copy(gt, gt_raw)

            stage_s = stage.tile([P, 4 * HM], mybir.dt.float32)
            stage_m = stage.tile([P, HM], mybir.dt.float32)
            stage_i = stage.tile([P, HM], mybir.dt.uint32)

            # Load order: T runs one heatmap ahead of X so the serial DVE
            # argmax chain (the critical path) starts as early as possible
            # while X still streams continuously behind it.
            t_tiles = {}
            x_tiles = {}

            # partition p holds 4 CONTIGUOUS heatmap rows [4p, 4p+4): each
            # partition is one 8KB contiguous DRAM run (4x fewer, 4x
            # bigger DMA descriptors than h-interleaving).  Loads are
            # split into halves so downstream ops can start on the first
            # half while the second streams (deps are range-accurate).
            def load_t(i):
                tt = tin.tile([P, TBLK, W], mybir.dt.float32)
                nc.sync.dma_start(out=tt, in_=t[i].rearrange("(p r) w -> p r w", p=P))
                t_tiles[i] = tt

            def load_x(i):
                xt = xin.tile([P, TBLK, W], mybir.dt.float32)
                nc.sync.dma_start(out=xt, in_=x[i].rearrange("(p r) w -> p r w", p=P))
                x_tiles[i] = xt

            load_t(0)
            load_t(1)
            for i in range(HM):
                load_x(i)
                if i + 2 < HM:
                    load_t(i + 2)

            for hh in range(HM):
                x_t = x_tiles[hh].rearrange("p t w -> p (t w)")
                t_t = t_tiles[hh].rearrange("p t w -> p (t w)")

                # --- softmax-sum path ---
                # bf16 E: 4x faster PE weight loads (FWL); G is exact in
                # bf16 and PSUM accumulates fp32, so only E's 2^-9
                # rounding enters the softmax sums (~1e-5 on pred coords)
                e_t = etp.tile([P, FREE], mybir.dt.bfloat16)
                nc.scalar.activation(e_t, x_t, mybir.ActivationFunctionType.Exp)

                # accumulate over 16 chunks: p1[m, c] = sum_j sum_p
                #   E[p, 128j+m] * g[p, j, c]
                p1 = ps1.tile([P, 4], mybir.dt.float32)
                for j in range(NCH):
                    nc.tensor.matmul(
                        p1,
                        e_t[:, P * j : P * (j + 1)],
                        gt[:, j, :],
                        start=(j == 0),
                        stop=(j == NCH - 1),
                    )
                nc.scalar.copy(stage_s[:, 4 * hh : 4 * hh + 4], p1)

                # --- argmax path ---
                m8 = small.tile([P, 8], mybir.dt.float32)
                nc.vector.max(out=m8, in_=t_t)
                i8 = small.tile([P, 8], mybir.dt.uint32)
                nc.vector.max_index(out=i8, in_max=m8, in_values=t_t)
                # keep DVE free for the max/max_index chain
                nc.scalar.copy(stage_m[:, hh : hh + 1], m8[:, 0:1])
                nc.gpsimd.tensor_copy(stage_i[:, hh : hh + 1], i8[:, 0:1])

            nc.sync.dma_start(out=sums[:, :], in_=stage_s)
            nc.sync.dma_start(out=maxv[:, :], in_=stage_m)
            nc.sync.dma_start(out=maxi[:, :], in_=stage_i)

    return nc


def _get_program():
    global _PROGRAM
    if _PROGRAM is None:
        _PROGRAM = _build_program()
        # hardware-only legalization; CoreSim's race detector can't
        # execute bare EventSemaphores, so keep it out of _build_program
        _split_multi_waits(_PROGRAM)
    return _PROGRAM


def _device_results(input, target, **run_kwargs):
    import ml_dtypes

    nc = _get_program()
    # heatmap row h = 4p + r, free index f = 512r + w (r = j//4 per chunk)
    # yg = (h+1)/512 = p/128 + (r+1)/512 ; xg = (w+1)/512 = (m+1)/512 + (j%4)/4
    gconst = np.empty((P, NCH, 4), dtype=np.float32)
    jj = np.arange(NCH, dtype=np.float32)
    gconst[:, :, 0] = 1.0
    gconst[:, :, 1] = (np.arange(P, dtype=np.float32) / float(P))[:, None]
    gconst[:, :, 2] = (jj % TBLK)[None, :] / float(TBLK)
    gconst[:, :, 3] = (np.floor(jj / TBLK)[None, :] + 1.0) / float(W)
    gconst = gconst.astype(ml_dtypes.bfloat16)
    in_maps = []
    for k in range(N_CORES):
        xs = np.ascontiguousarray(
            input[B_LOC * k : B_LOC * (k + 1)].reshape(HM, H, W), dtype=np.float32
        )
        ts = np.ascontiguousarray(
            target[B_LOC * k : B_LOC * (k + 1)].reshape(HM, H, W), dtype=np.float32
        )
        in_maps.append({"x": xs, "t": ts, "g": gconst})
    return run_bass_kernel_spmd(nc, in_maps, list(range(N_CORES)), **run_kwargs)


def _postprocess(results):
    px = np.zeros((B, C), dtype=np.float64)
    py = np.zeros((B, C), dtype=np.float64)
    tx = np.zeros((B, C), dtype=np.float64)
    ty = np.zeros((B, C), dtype=np.float64)
    mg = (np.arange(P, dtype=np.float64) + 1.0) / float(W)  # (m+1)/512
    for k in range(N_CORES):
        r = results[k]
        sums = r["sums"].astype(np.float64)      # [128, 4*8]
        maxv = r["maxv"]                         # [128, 8] f32
        maxi = r["maxi"].astype(np.int64)        # [128, 8]
        for hh in range(HM):
            b = B_LOC * k + hh // C
            c = hh % C
            p1 = sums[:, 4 * hh : 4 * hh + 4]    # [m, c]
            S = p1[:, 0].sum()
            Sx = (p1[:, 0] * mg).sum() + p1[:, 2].sum()
            Sy = p1[:, 1].sum() + p1[:, 3].sum()
            px[b, c] = Sx / S * W
            py[b, c] = Sy / S * H
            rm = maxv[:, hh]
            fi = maxi[:, hh]
            rows = np.nonzero(rm == rm.max())[0]
            flat = rows * (TBLK * W) + fi[rows]
            idx = int(flat.min())
            tx[b, c] = idx % W + 1
            ty[b, c] = idx // W + 1
    ed = np.sqrt((tx - px) ** 2 + (ty - py) ** 2)
    denom = float(B)
    s_i = ed[:, 0].sum() / denom
    s_s = ed[:, 1].sum() / denom
    s_total = s_i + s_s
    pred_diam = np.sqrt((px[:, 0] - px[:, 1]) ** 2 + (py[:, 0] - py[:, 1]) ** 2)
    true_diam = np.sqrt((tx[:, 0] - tx[:, 1]) ** 2 + (ty[:, 0] - ty[:, 1]) ** 2)
    s_diam_diff = (pred_diam - true_diam).sum() / denom
    pred_coords = (np.stack([px, py], axis=-1) - 1.0).astype(np.float32)
    true_coords = (np.stack([tx, ty], axis=-1) - 1.0).astype(np.float32)
    return (
        np.float32(s_i),
        np.float32(s_s),
        np.float32(s_total),
        np.float32(s_diam_diff),
        pred_coords,
        true_coords,
    )


def kernel(input, target):
    res = _device_results(np.asarray(input), np.asarray(target))
    return _postprocess(res.results)
